# revision 9
# baseline (speedup 1.0000x reference)
"""DifferentiablePooler on 8 TRN2 NeuronCores.

Reference computation (N=8192 nodes, NC=4096 coarse nodes, F=128 features):
    deg    = A.sum(-1)
    d      = rsqrt(max(deg, 1e-12))            (deg > 0 always here)
    H      = relu(diag(d) @ A @ diag(d) @ (X @ W_in) + b_in)    [N, F]
    Xp     = C @ H                                               [NC, F]
    out    = relu(Xp @ W_out + b_out)                            [NC, F]

Distribution: row-shard A (1024 rows/core) and C (512 rows/core) over 8 cores.
Per core, the A/C shards are pre-transposed on the host to [j, r] layout so the
tensor engine (which contracts over the partition dim of both operands) can
consume them without on-device transposes, and cast to fp16 so the 16 MB A
shard stays resident in SBUF (one HBM pass instead of two: one pass computes
deg, the matmul pass re-reads SBUF).

Device pipeline per core:
  XW   = X @ W_in                 (X^T streamed as lhsT, W_in as rhs)
  deg  = ones^T @ A_sh            (row sums via PE, M=1 matmuls)
  AllGather(deg) -> d = sqrt(1/deg) for all 8192 nodes
  Z^T  = sum_j (d*XW)[j-tile]^T-stationary @ A_sh[j-tile]      [F, 1024]
  H    = relu(d_own * Z + b_in)   (PE transpose to natural layout first)
  AllGather(H) -> full H [8192, 128]
  Xp^T = sum_j H[j-tile]-stationary @ C_sh[j-tile]             [F, 512]
  out^T= relu(W_out^T @ Xp^T + b_out)                          [F, 512]
Host gathers the 8 out^T shards and transposes into [4096, 128].
"""

import numpy as np

N = 8192
NCOARSE = 4096
F = 128
NCORES = 8
RPC = N // NCORES        # 1024 A-rows (nodes) per core
CPC = NCOARSE // NCORES  # 512 C-rows per core
JT = N // 128            # 64 contraction tiles
RT = RPC // 128          # 8 own-row tiles per core

_cached = {}


def _build():
    import concourse.bacc as bacc
    import concourse.tile as tile
    import concourse.mybir as mybir

    dt = mybir.dt
    f32, f16 = dt.float32, dt.float16
    Act = mybir.ActivationFunctionType
    Alu = mybir.AluOpType

    nc = bacc.Bacc("TRN2", target_bir_lowering=False, debug=False,
                   num_devices=NCORES)

    A_d = nc.dram_tensor("A_sh", [N, RPC], f16, kind="ExternalInput")
    C_d = nc.dram_tensor("C_sh", [N, CPC], f16, kind="ExternalInput")
    XT_d = nc.dram_tensor("X_T", [F, N], f16, kind="ExternalInput")
    Win_d = nc.dram_tensor("W_in16", [F, F], f16, kind="ExternalInput")
    Wout_d = nc.dram_tensor("W_out16", [F, F], f16, kind="ExternalInput")
    bin_d = nc.dram_tensor("b_in_bc", [128, F], f32, kind="ExternalInput")
    bout_d = nc.dram_tensor("b_out_c", [F, 1], f32, kind="ExternalInput")
    eye16_d = nc.dram_tensor("eye128_f16", [128, 128], f16, kind="ExternalInput")
    eye32_d = nc.dram_tensor("eye64_f32", [64, 64], f32, kind="ExternalInput")
    out_d = nc.dram_tensor("out_t", [F, CPC], f32, kind="ExternalOutput")

    groups = [list(range(NCORES))]

    with tile.TileContext(nc) as tc:
        with (
            tc.tile_pool(name="const", bufs=1) as cpool,
            tc.tile_pool(name="abig", bufs=1) as apool,
            tc.tile_pool(name="dram", bufs=1, space="DRAM") as dpool,
            tc.tile_pool(name="work", bufs=2) as work,
        ):
            # ---- constants ----
            win_sb = cpool.tile([F, F], f16)
            nc.sync.dma_start(win_sb[:], Win_d[:])
            wout_sb = cpool.tile([F, F], f16)
            nc.sync.dma_start(wout_sb[:], Wout_d[:])
            bin_sb = cpool.tile([128, F], f32)
            nc.sync.dma_start(bin_sb[:], bin_d[:])
            bout_sb = cpool.tile([F, 1], f32)
            nc.sync.dma_start(bout_sb[:], bout_d[:])
            eye16 = cpool.tile([128, 128], f16)
            nc.sync.dma_start(eye16[:], eye16_d[:])
            eye32 = cpool.tile([64, 64], f32)
            nc.sync.dma_start(eye32[:], eye32_d[:])
            ones = cpool.tile([128, 1], f16)
            nc.vector.memset(ones[:], 1.0)

            # ---- A shard resident in SBUF: [128, JT*RPC] fp16 (128 KB/part) ----
            A_big = apool.tile([128, JT * RPC], f16)
            for jc in range(JT):
                nc.sync.dma_start(
                    A_big[:, jc * RPC:(jc + 1) * RPC],
                    A_d[jc * 128:(jc + 1) * 128, :],
                )

            # ---- collective bounce buffers ----
            deg_in = dpool.tile([1, RPC], f32)          # local deg, node order
            deg_ag = dpool.tile([64, 128], f32)         # gathered deg, all nodes
            h_in = dpool.tile([RPC, F], f16)            # local H rows
            h_ag = dpool.tile([N, F], f16)

            # ---- XW = X @ W_in, fp16, natural layout [n, f] ----
            xw_sb = cpool.tile([128, JT * F], f16)
            with tc.tile_pool(name="xt", bufs=3) as xtpool, \
                 tc.tile_pool(name="pxw", bufs=2, space="PSUM") as pxwpool:
                for jc in range(JT):
                    xt_t = xtpool.tile([128, 128], f16)
                    nc.sync.dma_start(xt_t[:], XT_d[:, jc * 128:(jc + 1) * 128])
                    pxw = pxwpool.tile([128, F], f32)
                    nc.tensor.matmul(pxw[:], xt_t[:], win_sb[:])
                    nc.scalar.activation(
                        xw_sb[:, jc * F:(jc + 1) * F], pxw[:], Act.Copy)

            # ---- deg = row sums of own A rows (ones^T @ A_sh) ----
            with tc.tile_pool(name="pdeg", bufs=1, space="PSUM") as pdegpool:
                pdeg0 = pdegpool.tile([1, 512], f32)
                pdeg1 = pdegpool.tile([1, 512], f32)
                for jc in range(JT):
                    nc.tensor.matmul(
                        pdeg0[:], ones[:],
                        A_big[:, jc * RPC:jc * RPC + 512],
                        start=(jc == 0), stop=(jc == JT - 1))
                for jc in range(JT):
                    nc.tensor.matmul(
                        pdeg1[:], ones[:],
                        A_big[:, jc * RPC + 512:jc * RPC + 1024],
                        start=(jc == 0), stop=(jc == JT - 1))
                deg_sb = cpool.tile([1, RPC], f32)
                nc.vector.tensor_copy(deg_sb[:, 0:512], pdeg0[:])
                nc.vector.tensor_copy(deg_sb[:, 512:1024], pdeg1[:])

            nc.sync.dma_start(deg_in[:], deg_sb[:])

            # d_own = sqrt(1/deg_local) as [128, 8] (partition = node%128)
            with tc.tile_pool(name="ptd", bufs=1, space="PSUM") as ptdpool:
                t8 = cpool.tile([8, 128], f32)
                nc.sync.dma_start(
                    t8[:], deg_in[:].rearrange("a (b c) -> (a b) c", b=8))
                pt8 = ptdpool.tile([128, 8], f32)
                nc.tensor.transpose(pt8[:], t8[:], eye32[0:8, 0:8])
                rec8 = cpool.tile([128, 8], f32)
                nc.vector.reciprocal(rec8[:], pt8[:])
                d_own = cpool.tile([128, 8], f32)
                nc.scalar.activation(d_own[:], rec8[:], Act.Sqrt)

                # AllGather deg -> d_sb [128, 64] for all nodes
                nc.gpsimd.collective_compute(
                    "AllGather", Alu.bypass, replica_groups=groups,
                    ins=[deg_in.opt()], outs=[deg_ag.opt()])
                t64 = cpool.tile([64, 128], f32)
                nc.sync.dma_start(t64[:], deg_ag[:])
                pt64 = ptdpool.tile([128, 64], f32)
                nc.tensor.transpose(pt64[:], t64[:], eye32[:])
                rec64 = cpool.tile([128, 64], f32)
                nc.vector.reciprocal(rec64[:], pt64[:])
                d_sb = cpool.tile([128, 64], f32)
                nc.scalar.activation(d_sb[:], rec64[:], Act.Sqrt)

            # ---- stage 1: Z^T = sum_j Y[j]^T-stationary @ A_sh[j] ----
            with tc.tile_pool(name="ypool", bufs=3) as ypool, \
                 tc.tile_pool(name="pz", bufs=1, space="PSUM") as pzpool:
                pz0 = pzpool.tile([128, 512], f32)
                pz1 = pzpool.tile([128, 512], f32)
                for jc in range(JT):
                    y_t = ypool.tile([128, F], f16)
                    nc.vector.tensor_scalar_mul(
                        y_t[:], xw_sb[:, jc * F:(jc + 1) * F],
                        d_sb[:, jc:jc + 1])
                    nc.tensor.matmul(
                        pz0[:], y_t[:], A_big[:, jc * RPC:jc * RPC + 512],
                        start=(jc == 0), stop=(jc == JT - 1))
                    nc.tensor.matmul(
                        pz1[:], y_t[:], A_big[:, jc * RPC + 512:jc * RPC + 1024],
                        start=(jc == 0), stop=(jc == JT - 1))

                # ---- H = relu(d_own * Z + b_in), natural layout, to DRAM ----
                with tc.tile_pool(name="ptr", bufs=2, space="PSUM") as ptrpool, \
                     tc.tile_pool(name="hloc", bufs=3) as hpool:
                    for rcb in range(2):
                        pz = pz0 if rcb == 0 else pz1
                        zt_sb = work.tile([128, 512], f16, tag="zt")
                        nc.scalar.activation(zt_sb[:], pz[:], Act.Copy)
                        for sub in range(4):
                            rci = rcb * 4 + sub
                            ptr = ptrpool.tile([128, 128], f16)
                            nc.tensor.transpose(
                                ptr[:], zt_sb[:, sub * 128:(sub + 1) * 128],
                                eye16[:])
                            tmp = hpool.tile([128, F], f32, tag="tmp")
                            nc.vector.scalar_tensor_tensor(
                                tmp[:], ptr[:], d_own[:, rci:rci + 1],
                                bin_sb[:], Alu.mult, Alu.add)
                            h_t = hpool.tile([128, F], f16, tag="h")
                            nc.vector.tensor_scalar_max(h_t[:], tmp[:], 0.0)
                            nc.sync.dma_start(
                                h_in[rci * 128:(rci + 1) * 128, :], h_t[:])

            # ---- AllGather H ----
            nc.gpsimd.collective_compute(
                "AllGather", Alu.bypass, replica_groups=groups,
                ins=[h_in.opt()], outs=[h_ag.opt()])

            # ---- stage 2: Xp^T = sum_j H[j]-stationary @ C_sh[j] ----
            with tc.tile_pool(name="hg", bufs=8) as hgpool, \
                 tc.tile_pool(name="cs", bufs=8) as cspool, \
                 tc.tile_pool(name="pxp", bufs=1, space="PSUM") as pxppool:
                pxp = pxppool.tile([128, CPC], f32)
                for jc in range(JT):
                    c_t = cspool.tile([128, CPC], f16)
                    nc.sync.dma_start(c_t[:], C_d[jc * 128:(jc + 1) * 128, :])
                    h_l = hgpool.tile([128, F], f16)
                    nc.sync.dma_start(h_l[:], h_ag[jc * 128:(jc + 1) * 128, :])
                    nc.tensor.matmul(pxp[:], h_l[:], c_t[:],
                                     start=(jc == 0), stop=(jc == JT - 1))

                # ---- out^T = relu(W_out^T @ Xp^T + b_out) ----
                xp_sb = work.tile([128, CPC], f16, tag="xp")
                nc.scalar.activation(xp_sb[:], pxp[:], Act.Copy)
                pout = pxppool.tile([128, CPC], f32)
                nc.tensor.matmul(pout[:], wout_sb[:], xp_sb[:])
                out_sb = work.tile([128, CPC], f32, tag="out")
                nc.scalar.activation(out_sb[:], pout[:], Act.Relu,
                                     bias=bout_sb[:, 0:1])
                nc.sync.dma_start(out_d[:], out_sb[:])

    nc.compile()
    return nc


def _prep_in_maps(X, A, C, W_in, b_in, W_out, b_out):
    X_T = np.ascontiguousarray(X.T).astype(np.float16)
    W_in16 = W_in.astype(np.float16)
    W_out16 = W_out.astype(np.float16)
    b_in_bc = np.tile(np.asarray(b_in, np.float32)[None, :], (128, 1))
    b_out_c = np.ascontiguousarray(np.asarray(b_out, np.float32)[:, None])
    eye128_f16 = np.eye(128, dtype=np.float16)
    eye64_f32 = np.eye(64, dtype=np.float32)

    in_maps = []
    for k in range(NCORES):
        A_sh = np.ascontiguousarray(A[k * RPC:(k + 1) * RPC, :].T).astype(
            np.float16)
        C_sh = np.ascontiguousarray(C[k * CPC:(k + 1) * CPC, :].T).astype(
            np.float16)
        in_maps.append({
            "A_sh": A_sh, "C_sh": C_sh, "X_T": X_T,
            "W_in16": W_in16, "W_out16": W_out16,
            "b_in_bc": b_in_bc, "b_out_c": b_out_c,
            "eye128_f16": eye128_f16, "eye64_f32": eye64_f32,
        })
    return in_maps


def _get_nc():
    if "nc" not in _cached:
        _cached["nc"] = _build()
    return _cached["nc"]


def kernel(X, A, C, W_in, b_in, W_out, b_out):
    from concourse.bass_utils import run_bass_kernel_spmd

    nc = _get_nc()
    in_maps = _prep_in_maps(X, A, C, W_in, b_in, W_out, b_out)
    res = run_bass_kernel_spmd(nc, in_maps, core_ids=list(range(NCORES)))
    out = np.concatenate(
        [np.asarray(res.results[k]["out_t"]).T for k in range(NCORES)], axis=0)
    return np.ascontiguousarray(out.astype(np.float32))


def profile_run(inputs):
    """Amortized wall-clock of the compiled NEFF with device-resident inputs.

    The axon NTFF profile hook is unavailable in this container, so this is
    an upper bound on HW exec time: dispatch + NEFF execution, inputs already
    on device, averaged over repeats.
    """
    import time as _t
    import jax
    from jax.sharding import Mesh, NamedSharding, PartitionSpec
    from jax.experimental.shard_map import shard_map
    import concourse.mybir as mybir
    from concourse import bass2jax

    nc = _get_nc()
    in_maps = _prep_in_maps(**inputs)
    bass2jax.install_neuronx_cc_hook()
    pname = nc.partition_id_tensor.name if nc.partition_id_tensor else None
    in_names, out_names, out_avals, zero_outs = [], [], [], []
    for alloc in nc.m.functions[0].allocations:
        if not isinstance(alloc, mybir.MemoryLocationSet):
            continue
        name = alloc.memorylocations[0].name
        if alloc.kind == "ExternalInput":
            if name != pname:
                in_names.append(name)
        elif alloc.kind == "ExternalOutput":
            out_names.append(name)
            shape = tuple(alloc.tensor_shape)
            dtype = mybir.dt.np(alloc.dtype)
            out_avals.append(jax.core.ShapedArray(shape, dtype))
            zero_outs.append(np.zeros(shape, dtype))
    n_params, n_outs = len(in_names), len(out_avals)
    all_names = list(in_names) + out_names + ([pname] if pname else [])

    def _body(*args):
        operands = list(args)
        if pname is not None:
            operands.append(bass2jax.partition_id_tensor())
        outs = bass2jax._bass_exec_p.bind(
            *operands, out_avals=tuple(out_avals), in_names=tuple(all_names),
            out_names=tuple(out_names), lowering_input_output_aliases=(),
            sim_require_finite=True, sim_require_nnan=True, nc=nc)
        return tuple(outs)

    devices = jax.devices()[:NCORES]
    mesh = Mesh(np.asarray(devices), ("core",))
    sharded = jax.jit(
        shard_map(_body, mesh=mesh,
                  in_specs=(PartitionSpec("core"),) * (n_params + n_outs),
                  out_specs=(PartitionSpec("core"),) * n_outs,
                  check_rep=False),
        donate_argnums=tuple(range(n_params, n_params + n_outs)),
        keep_unused=True)
    sh = NamedSharding(mesh, PartitionSpec("core"))
    dev_in = [
        jax.device_put(
            np.concatenate([np.asarray(in_maps[c][n]) for c in range(NCORES)],
                           axis=0), sh)
        for n in in_names]

    def zeros():
        return [jax.device_put(
            np.zeros((NCORES * z.shape[0], *z.shape[1:]), z.dtype), sh)
            for z in zero_outs]

    r = sharded(*dev_in, *zeros())
    jax.block_until_ready(r)
    REP = 10
    zs = [zeros() for _ in range(REP)]
    t0 = _t.perf_counter()
    for i in range(REP):
        r = sharded(*dev_in, *zs[i])
    jax.block_until_ready(r)
    return int((_t.perf_counter() - t0) / REP * 1e9)


# revision 11
# speedup vs baseline: 1.0880x; 1.0880x over previous
"""DifferentiablePooler on 8 TRN2 NeuronCores.

Reference computation (N=8192 nodes, NC=4096 coarse nodes, F=128 features):
    deg    = A.sum(-1)
    d      = rsqrt(max(deg, 1e-12))            (deg > 0 always here)
    H      = relu(diag(d) @ A @ diag(d) @ (X @ W_in) + b_in)    [N, F]
    Xp     = C @ H                                               [NC, F]
    out    = relu(Xp @ W_out + b_out)                            [NC, F]

Distribution: row-shard A (1024 rows/core) and C (512 rows/core) over 8 cores.
Per core, the A/C shards are pre-transposed on the host to [j, r] layout so the
tensor engine (which contracts over the partition dim of both operands) can
consume them without on-device transposes, and cast to fp16 so the 16 MB A
shard stays resident in SBUF (one HBM pass instead of two: one pass computes
deg, the matmul pass re-reads SBUF).

Device pipeline per core:
  XW   = X @ W_in                 (X^T streamed as lhsT, W_in as rhs)
  deg  = ones^T @ A_sh            (row sums via PE, M=1 matmuls)
  AllGather(deg) -> d = sqrt(1/deg) for all 8192 nodes
  Z^T  = sum_j (d*XW)[j-tile]^T-stationary @ A_sh[j-tile]      [F, 1024]
  H    = relu(d_own * Z + b_in)   (PE transpose to natural layout first)
  AllGather(H) -> full H [8192, 128]
  Xp^T = sum_j H[j-tile]-stationary @ C_sh[j-tile]             [F, 512]
  out^T= relu(W_out^T @ Xp^T + b_out)                          [F, 512]
Host gathers the 8 out^T shards and transposes into [4096, 128].
"""

import numpy as np

N = 8192
NCOARSE = 4096
F = 128
NCORES = 8
RPC = N // NCORES        # 1024 A-rows (nodes) per core
CPC = NCOARSE // NCORES  # 512 C-rows per core
JT = N // 128            # 64 contraction tiles
RT = RPC // 128          # 8 own-row tiles per core

_cached = {}


def _build():
    import concourse.bacc as bacc
    import concourse.tile as tile
    import concourse.mybir as mybir

    dt = mybir.dt
    f32, f16 = dt.float32, dt.float16
    Act = mybir.ActivationFunctionType
    Alu = mybir.AluOpType

    nc = bacc.Bacc("TRN2", target_bir_lowering=False, debug=False,
                   num_devices=NCORES)

    A_d = nc.dram_tensor("A_sh", [N, RPC], f16, kind="ExternalInput")
    C_d = nc.dram_tensor("C_sh", [N, CPC], f16, kind="ExternalInput")
    XT_d = nc.dram_tensor("X_T", [F, N], f16, kind="ExternalInput")
    Win_d = nc.dram_tensor("W_in16", [F, F], f16, kind="ExternalInput")
    Wout_d = nc.dram_tensor("W_out16", [F, F], f16, kind="ExternalInput")
    bin_d = nc.dram_tensor("b_in_bc", [128, F], f32, kind="ExternalInput")
    bout_d = nc.dram_tensor("b_out_c", [F, 1], f32, kind="ExternalInput")
    eye16_d = nc.dram_tensor("eye128_f16", [128, 128], f16, kind="ExternalInput")
    eye32_d = nc.dram_tensor("eye64_f32", [64, 64], f32, kind="ExternalInput")
    out_d = nc.dram_tensor("out_t", [F, CPC], f32, kind="ExternalOutput")

    groups = [list(range(NCORES))]

    with tile.TileContext(nc) as tc:
        with (
            tc.tile_pool(name="const", bufs=1) as cpool,
            tc.tile_pool(name="abig", bufs=1) as apool,
            tc.tile_pool(name="dram", bufs=1, space="DRAM") as dpool,
            tc.tile_pool(name="work", bufs=2) as work,
        ):
            # ---- constants ----
            win_sb = cpool.tile([F, F], f16)
            nc.sync.dma_start(win_sb[:], Win_d[:])
            wout_sb = cpool.tile([F, F], f16)
            nc.sync.dma_start(wout_sb[:], Wout_d[:])
            bin_sb = cpool.tile([128, F], f32)
            nc.sync.dma_start(bin_sb[:], bin_d[:])
            bout_sb = cpool.tile([F, 1], f32)
            nc.sync.dma_start(bout_sb[:], bout_d[:])
            eye16 = cpool.tile([128, 128], f16)
            nc.sync.dma_start(eye16[:], eye16_d[:])
            eye32 = cpool.tile([64, 64], f32)
            nc.sync.dma_start(eye32[:], eye32_d[:])
            ones = cpool.tile([128, 1], f16)
            nc.vector.memset(ones[:], 1.0)

            # ---- A shard resident in SBUF: [128, JT*RPC] fp16 (128 KB/part) ----
            A_big = apool.tile([128, JT * RPC], f16)
            for jc in range(JT):
                nc.sync.dma_start(
                    A_big[:, jc * RPC:(jc + 1) * RPC],
                    A_d[jc * 128:(jc + 1) * 128, :],
                )

            # ---- collective bounce buffers ----
            deg_in = dpool.tile([1, RPC], f32)          # local deg, node order
            deg_ag = dpool.tile([64, 128], f32, addr_space="Shared")
            h_in = dpool.tile([RPC, F], f16)            # local H rows
            h_ag = dpool.tile([N, F], f16, addr_space="Shared")

            # ---- XW = X @ W_in, fp16, natural layout [n, f] ----
            xw_sb = cpool.tile([128, JT * F], f16)
            with tc.tile_pool(name="xt", bufs=3) as xtpool, \
                 tc.tile_pool(name="pxw", bufs=2, space="PSUM") as pxwpool:
                for jc in range(JT):
                    xt_t = xtpool.tile([128, 128], f16)
                    nc.sync.dma_start(xt_t[:], XT_d[:, jc * 128:(jc + 1) * 128])
                    pxw = pxwpool.tile([128, F], f32)
                    nc.tensor.matmul(pxw[:], xt_t[:], win_sb[:])
                    nc.scalar.activation(
                        xw_sb[:, jc * F:(jc + 1) * F], pxw[:], Act.Copy)

            # ---- deg = row sums of own A rows (ones^T @ A_sh) ----
            with tc.tile_pool(name="pdeg", bufs=1, space="PSUM") as pdegpool:
                pdeg0 = pdegpool.tile([1, 512], f32)
                pdeg1 = pdegpool.tile([1, 512], f32)
                for jc in range(JT):
                    nc.tensor.matmul(
                        pdeg0[:], ones[:],
                        A_big[:, jc * RPC:jc * RPC + 512],
                        start=(jc == 0), stop=(jc == JT - 1))
                for jc in range(JT):
                    nc.tensor.matmul(
                        pdeg1[:], ones[:],
                        A_big[:, jc * RPC + 512:jc * RPC + 1024],
                        start=(jc == 0), stop=(jc == JT - 1))
                deg_sb = cpool.tile([1, RPC], f32)
                nc.vector.tensor_copy(deg_sb[:, 0:512], pdeg0[:])
                nc.vector.tensor_copy(deg_sb[:, 512:1024], pdeg1[:])

            nc.sync.dma_start(deg_in[:], deg_sb[:])

            # d_own = sqrt(1/deg_local) as [128, 8] (partition = node%128)
            with tc.tile_pool(name="ptd", bufs=1, space="PSUM") as ptdpool:
                t8 = cpool.tile([8, 128], f32)
                nc.sync.dma_start(
                    t8[:], deg_in[:].rearrange("a (b c) -> (a b) c", b=8))
                pt8 = ptdpool.tile([128, 8], f32)
                nc.tensor.transpose(pt8[:], t8[:], eye32[0:8, 0:8])
                rec8 = cpool.tile([128, 8], f32)
                nc.vector.reciprocal(rec8[:], pt8[:])
                d_own = cpool.tile([128, 8], f32)
                nc.scalar.activation(d_own[:], rec8[:], Act.Sqrt)

                # AllGather deg -> d_sb [128, 64] for all nodes
                nc.gpsimd.collective_compute(
                    "AllGather", Alu.bypass, replica_groups=groups,
                    ins=[deg_in.opt()], outs=[deg_ag.opt()])
                t64 = cpool.tile([64, 128], f32)
                nc.sync.dma_start(t64[:], deg_ag[:])
                pt64 = ptdpool.tile([128, 64], f32)
                nc.tensor.transpose(pt64[:], t64[:], eye32[:])
                rec64 = cpool.tile([128, 64], f32)
                nc.vector.reciprocal(rec64[:], pt64[:])
                d_sb = cpool.tile([128, 64], f32)
                nc.scalar.activation(d_sb[:], rec64[:], Act.Sqrt)

            # ---- stage 1: Z^T = sum_j Y[j]^T-stationary @ A_sh[j] ----
            with tc.tile_pool(name="ypool", bufs=3) as ypool, \
                 tc.tile_pool(name="pz", bufs=1, space="PSUM") as pzpool:
                pz0 = pzpool.tile([128, 512], f32)
                pz1 = pzpool.tile([128, 512], f32)
                for jc in range(JT):
                    y_t = ypool.tile([128, F], f16)
                    nc.vector.tensor_scalar_mul(
                        y_t[:], xw_sb[:, jc * F:(jc + 1) * F],
                        d_sb[:, jc:jc + 1])
                    nc.tensor.matmul(
                        pz0[:], y_t[:], A_big[:, jc * RPC:jc * RPC + 512],
                        start=(jc == 0), stop=(jc == JT - 1))
                    nc.tensor.matmul(
                        pz1[:], y_t[:], A_big[:, jc * RPC + 512:jc * RPC + 1024],
                        start=(jc == 0), stop=(jc == JT - 1))

                # ---- H = relu(d_own * Z + b_in), natural layout, to DRAM ----
                with tc.tile_pool(name="ptr", bufs=2, space="PSUM") as ptrpool, \
                     tc.tile_pool(name="hloc", bufs=3) as hpool:
                    for rcb in range(2):
                        pz = pz0 if rcb == 0 else pz1
                        zt_sb = work.tile([128, 512], f16, tag="zt")
                        nc.scalar.activation(zt_sb[:], pz[:], Act.Copy)
                        for sub in range(4):
                            rci = rcb * 4 + sub
                            ptr = ptrpool.tile([128, 128], f16)
                            nc.tensor.transpose(
                                ptr[:], zt_sb[:, sub * 128:(sub + 1) * 128],
                                eye16[:])
                            tmp = hpool.tile([128, F], f32, tag="tmp")
                            nc.vector.scalar_tensor_tensor(
                                tmp[:], ptr[:], d_own[:, rci:rci + 1],
                                bin_sb[:], Alu.mult, Alu.add)
                            h_t = hpool.tile([128, F], f16, tag="h")
                            nc.vector.tensor_scalar_max(h_t[:], tmp[:], 0.0)
                            nc.sync.dma_start(
                                h_in[rci * 128:(rci + 1) * 128, :], h_t[:])

            # ---- AllGather H ----
            nc.gpsimd.collective_compute(
                "AllGather", Alu.bypass, replica_groups=groups,
                ins=[h_in.opt()], outs=[h_ag.opt()])

            # ---- stage 2: Xp^T = sum_j H[j]-stationary @ C_sh[j] ----
            with tc.tile_pool(name="hg", bufs=8) as hgpool, \
                 tc.tile_pool(name="cs", bufs=8) as cspool, \
                 tc.tile_pool(name="pxp", bufs=1, space="PSUM") as pxppool:
                pxp = pxppool.tile([128, CPC], f32)
                for jc in range(JT):
                    c_t = cspool.tile([128, CPC], f16)
                    nc.sync.dma_start(c_t[:], C_d[jc * 128:(jc + 1) * 128, :])
                    h_l = hgpool.tile([128, F], f16)
                    nc.sync.dma_start(h_l[:], h_ag[jc * 128:(jc + 1) * 128, :])
                    nc.tensor.matmul(pxp[:], h_l[:], c_t[:],
                                     start=(jc == 0), stop=(jc == JT - 1))

                # ---- out^T = relu(W_out^T @ Xp^T + b_out) ----
                xp_sb = work.tile([128, CPC], f16, tag="xp")
                nc.scalar.activation(xp_sb[:], pxp[:], Act.Copy)
                pout = pxppool.tile([128, CPC], f32)
                nc.tensor.matmul(pout[:], wout_sb[:], xp_sb[:])
                out_sb = work.tile([128, CPC], f32, tag="out")
                nc.scalar.activation(out_sb[:], pout[:], Act.Relu,
                                     bias=bout_sb[:, 0:1])
                nc.sync.dma_start(out_d[:], out_sb[:])

    nc.compile()
    return nc


def _prep_in_maps(X, A, C, W_in, b_in, W_out, b_out):
    X_T = np.ascontiguousarray(X.T).astype(np.float16)
    W_in16 = W_in.astype(np.float16)
    W_out16 = W_out.astype(np.float16)
    b_in_bc = np.tile(np.asarray(b_in, np.float32)[None, :], (128, 1))
    b_out_c = np.ascontiguousarray(np.asarray(b_out, np.float32)[:, None])
    eye128_f16 = np.eye(128, dtype=np.float16)
    eye64_f32 = np.eye(64, dtype=np.float32)

    in_maps = []
    for k in range(NCORES):
        A_sh = np.ascontiguousarray(A[k * RPC:(k + 1) * RPC, :].T).astype(
            np.float16)
        C_sh = np.ascontiguousarray(C[k * CPC:(k + 1) * CPC, :].T).astype(
            np.float16)
        in_maps.append({
            "A_sh": A_sh, "C_sh": C_sh, "X_T": X_T,
            "W_in16": W_in16, "W_out16": W_out16,
            "b_in_bc": b_in_bc, "b_out_c": b_out_c,
            "eye128_f16": eye128_f16, "eye64_f32": eye64_f32,
        })
    return in_maps


def _get_nc():
    if "nc" not in _cached:
        _cached["nc"] = _build()
    return _cached["nc"]


def kernel(X, A, C, W_in, b_in, W_out, b_out):
    from concourse.bass_utils import run_bass_kernel_spmd

    nc = _get_nc()
    in_maps = _prep_in_maps(X, A, C, W_in, b_in, W_out, b_out)
    res = run_bass_kernel_spmd(nc, in_maps, core_ids=list(range(NCORES)))
    out = np.concatenate(
        [np.asarray(res.results[k]["out_t"]).T for k in range(NCORES)], axis=0)
    return np.ascontiguousarray(out.astype(np.float32))


def profile_run(inputs):
    """Amortized wall-clock of the compiled NEFF with device-resident inputs.

    The axon NTFF profile hook is unavailable in this container, so this is
    an upper bound on HW exec time: dispatch + NEFF execution, inputs already
    on device, averaged over repeats.
    """
    import time as _t
    import jax
    from jax.sharding import Mesh, NamedSharding, PartitionSpec
    from jax.experimental.shard_map import shard_map
    import concourse.mybir as mybir
    from concourse import bass2jax

    nc = _get_nc()
    in_maps = _prep_in_maps(**inputs)
    bass2jax.install_neuronx_cc_hook()
    pname = nc.partition_id_tensor.name if nc.partition_id_tensor else None
    in_names, out_names, out_avals, zero_outs = [], [], [], []
    for alloc in nc.m.functions[0].allocations:
        if not isinstance(alloc, mybir.MemoryLocationSet):
            continue
        name = alloc.memorylocations[0].name
        if alloc.kind == "ExternalInput":
            if name != pname:
                in_names.append(name)
        elif alloc.kind == "ExternalOutput":
            out_names.append(name)
            shape = tuple(alloc.tensor_shape)
            dtype = mybir.dt.np(alloc.dtype)
            out_avals.append(jax.core.ShapedArray(shape, dtype))
            zero_outs.append(np.zeros(shape, dtype))
    n_params, n_outs = len(in_names), len(out_avals)
    all_names = list(in_names) + out_names + ([pname] if pname else [])

    def _body(*args):
        operands = list(args)
        if pname is not None:
            operands.append(bass2jax.partition_id_tensor())
        outs = bass2jax._bass_exec_p.bind(
            *operands, out_avals=tuple(out_avals), in_names=tuple(all_names),
            out_names=tuple(out_names), lowering_input_output_aliases=(),
            sim_require_finite=True, sim_require_nnan=True, nc=nc)
        return tuple(outs)

    devices = jax.devices()[:NCORES]
    mesh = Mesh(np.asarray(devices), ("core",))
    sharded = jax.jit(
        shard_map(_body, mesh=mesh,
                  in_specs=(PartitionSpec("core"),) * (n_params + n_outs),
                  out_specs=(PartitionSpec("core"),) * n_outs,
                  check_rep=False),
        donate_argnums=tuple(range(n_params, n_params + n_outs)),
        keep_unused=True)
    sh = NamedSharding(mesh, PartitionSpec("core"))
    dev_in = [
        jax.device_put(
            np.concatenate([np.asarray(in_maps[c][n]) for c in range(NCORES)],
                           axis=0), sh)
        for n in in_names]

    def zeros():
        return [jax.device_put(
            np.zeros((NCORES * z.shape[0], *z.shape[1:]), z.dtype), sh)
            for z in zero_outs]

    r = sharded(*dev_in, *zeros())
    jax.block_until_ready(r)
    REP = 10
    zs = [zeros() for _ in range(REP)]
    t0 = _t.perf_counter()
    for i in range(REP):
        r = sharded(*dev_in, *zs[i])
    jax.block_until_ready(r)
    return int((_t.perf_counter() - t0) / REP * 1e9)
